# revision 19
# baseline (speedup 1.0000x reference)
"""BertInsertion loss kernel for 8 Trainium2 NeuronCores.

Strategy: pure data parallelism over the batch (64 rows -> 8 rows/core).
Each core, on device:
  1. Finds the C=16 marked positions per row from the 0/1 sot mask
     (bf16 cumsum scan + fused count-compare, no sort).
  2. Ragged-gathers the 16 D=1024 vectors per row from HBM with an
     indirect DMA (~0.5 MB instead of the 16 MB shard); the 8 speaker
     rows are a second tiny gather issued first, then broadcast to
     their 16 partitions on the idle tensor engine.
  3. Cosine sims via multiply + row-reduce; 1/(|a||sp|) from the scalar
     engine's Sqrt (pre-warmed, stays resident: Square is table-free)
     and the exact vector-engine reciprocal.
  4. Per-row softmax statistics (sum of exps, label-selected sim) and
     the argmax prediction, packed into one [8,3] output.
Host reshards inputs, supplies input-independent constants, and runs the
loss epilogue: ce = ln(sumexp) - sim_label, then the masked mean.
"""

import numpy as np
import ml_dtypes

import concourse.bass as bass
import concourse.bacc as bacc
import concourse.tile as tile
from concourse import mybir
from concourse.bass_utils import run_bass_kernel_spmd

B, S, D, C = 64, 512, 1024, 16
NCORES = 8
BL = B // NCORES  # batch rows per core
P = BL * C        # 128 gathered vectors per core = full partition dim
N12MIN = 1e-12    # clamp for |a|*|sp|, matches eps=1e-6 per norm
NEG = -40.0       # below any cosine sim; exp(-40) ~ 4e-18, invisible in f32

F32 = mybir.dt.float32
BF16 = mybir.dt.bfloat16
I32 = mybir.dt.int32
U32 = mybir.dt.uint32
Op = mybir.AluOpType
Act = mybir.ActivationFunctionType

# fbig [128, 171] f32:
#   onehotPT(16) | ET(8) | rvals(1) | offv(1) | it16(16, rows 0:8) |
#   E(128, rows 0:8) | spioff(1, rows 0:8)
FB_W = C + BL + 1 + 1 + C + P + 1
# mbig [8, 641] bf16: maskf(512) | E(128) | labels+1(1)
MB_W = S + P + 1


def _make_consts():
    p = np.arange(P)
    b = np.arange(BL)
    onehotPT = (p[:, None] % C == np.arange(C)[None, :]).astype(np.float32)
    ET = (p[:, None] // C == b[None, :]).astype(np.float32)
    rvals = (p % C + 1).astype(np.float32)[:, None]
    offv = ((p // C) * S).astype(np.float32)[:, None]
    it16 = np.zeros((P, C), np.float32)
    it16[:BL] = np.arange(C, dtype=np.float32)[None, :]
    E = (np.arange(P)[None, :] // C == b[:, None]).astype(np.float32)
    E_pad = np.zeros((P, P), np.float32)
    E_pad[:BL] = E
    spioff = np.zeros((P, 1), np.float32)
    spioff[:BL, 0] = b * S
    fbig = np.ascontiguousarray(
        np.concatenate([onehotPT, ET, rvals, offv, it16, E_pad, spioff],
                       axis=1))
    ebf = E.astype(ml_dtypes.bfloat16)
    return fbig, ebf


def _build():
    nc = bacc.Bacc("TRN2", target_bir_lowering=False, debug=False)

    seq = nc.dram_tensor("seq", [BL * S, D], F32, kind="ExternalInput").ap()
    mbig_d = nc.dram_tensor("mbig", [BL, MB_W], BF16, kind="ExternalInput").ap()
    fbig_d = nc.dram_tensor("fbig", [P, FB_W], F32, kind="ExternalInput").ap()
    out_d = nc.dram_tensor("out", [BL, 3], F32, kind="ExternalOutput").ap()

    with tile.TileContext(nc) as tc:
        import contextlib

        with contextlib.ExitStack() as ctx:
            pool = ctx.enter_context(tc.tile_pool(name="sb", bufs=1))
            psum = ctx.enter_context(tc.tile_pool(name="ps", bufs=1, space="PSUM"))

            # -------- input DMAs: one per HWDGE queue ---------------------
            mbig = pool.tile([BL, MB_W], BF16)
            nc.sync.dma_start(out=mbig[:], in_=mbig_d)
            fbig = pool.tile([P, FB_W], F32)
            nc.scalar.dma_start(out=fbig[:], in_=fbig_d)

            m_bf = mbig[:, 0:S]
            ebf = mbig[:, S:S + P]
            lp1 = mbig[:, S + P:S + P + 1]
            c0 = 0
            onehotPT = fbig[:, c0:c0 + C]
            c0 += C
            ET = fbig[:, c0:c0 + BL]
            c0 += BL
            rvals = fbig[:, c0:c0 + 1]
            c0 += 1
            offv = fbig[:, c0:c0 + 1]
            c0 += 1
            it16 = fbig[0:BL, c0:c0 + C]
            c0 += C
            E = fbig[0:BL, c0:c0 + P]
            c0 += P
            spioff = fbig[0:BL, c0:c0 + 1]

            # -------- pre-warm Sqrt (stays resident: Square is tableless) -
            w = pool.tile([1, 1], F32)
            nc.vector.memset(w[:], 1.0)
            wo = pool.tile([1, 3], F32)
            nc.scalar.activation(out=wo[:, 0:1], in_=w[:], func=Act.Sqrt)

            # -------- stage A: mask -> gather indices ---------------------
            zeros = pool.tile([BL, S], BF16)
            nc.vector.memset(zeros[:], 0.0)
            cum = pool.tile([BL, S], BF16)  # inclusive cumsum (values <= 16)
            nc.vector.tensor_tensor_scan(out=cum[:], data0=m_bf,
                                         data1=zeros[:], initial=0.0,
                                         op0=Op.add, op1=Op.add)
            # speaker (first marked position): tiny gather issued first
            spcnt = pool.tile([BL, S], F32)
            spidx_f = pool.tile([BL, 1], F32)
            nc.vector.tensor_scalar(out=spcnt[:], in0=cum[:], scalar1=1.0,
                                    scalar2=None, op0=Op.is_lt, op1=Op.add,
                                    accum_out=spidx_f[:])
            spidx_i = pool.tile([BL, 1], I32)
            nc.vector.tensor_scalar(out=spidx_i[:], in0=spidx_f[:],
                                    scalar1=spioff, scalar2=None, op0=Op.add)
            Spk = pool.tile([BL, D], F32)
            nc.gpsimd.indirect_dma_start(
                out=Spk[:], out_offset=None, in_=seq,
                in_offset=bass.IndirectOffsetOnAxis(ap=spidx_i[:, :1], axis=0))

            cumB = psum.tile([P, S], F32)
            nc.tensor.matmul(out=cumB[:], lhsT=ebf, rhs=cum[:],
                             start=True, stop=True)
            ind = pool.tile([P, S], F32)
            idx_f = pool.tile([P, 1], F32)
            nc.vector.tensor_scalar(out=ind[:], in0=cumB[:], scalar1=rvals,
                                    scalar2=None, op0=Op.is_lt, op1=Op.add,
                                    accum_out=idx_f[:])
            idx_i = pool.tile([P, 1], I32)
            nc.vector.tensor_scalar(out=idx_i[:], in0=idx_f[:], scalar1=offv,
                                    scalar2=None, op0=Op.add)
            G = pool.tile([P, D], F32)
            nc.gpsimd.indirect_dma_start(
                out=G[:], out_offset=None, in_=seq,
                in_offset=bass.IndirectOffsetOnAxis(ap=idx_i[:, :1], axis=0))

            # label one-hot, ready long before the tail
            lp1f = pool.tile([BL, 1], F32)
            nc.vector.tensor_copy(out=lp1f[:], in_=lp1)
            onehotF = pool.tile([BL, C], F32)
            nc.vector.tensor_scalar(out=onehotF[:], in0=it16, scalar1=lp1f[:],
                                    scalar2=None, op0=Op.is_equal)

            # -------- stage C: speaker norms + broadcast (all pre-G) ------
            sqsp = pool.tile([BL, D], F32)
            n2sp = pool.tile([BL, 1], F32)
            nc.scalar.activation(out=sqsp[:], in_=Spk[:], func=Act.Square,
                                 accum_out=n2sp[:])
            n2spB = psum.tile([P, 1], F32)
            nc.tensor.matmul(out=n2spB[:], lhsT=E, rhs=n2sp[:],
                             start=True, stop=True)
            SpB = psum.tile([P, D], F32)
            for h in range(2):
                cs = slice(h * 512, (h + 1) * 512)
                nc.tensor.matmul(out=SpB[:, cs], lhsT=E, rhs=Spk[:, cs],
                                 start=True, stop=True)

            # -------- main norms + sims (post-G) --------------------------
            sq = pool.tile([P, D], F32)
            n2 = pool.tile([P, 1], F32)
            nc.scalar.activation(out=sq[:], in_=G[:], func=Act.Square,
                                 accum_out=n2[:])
            dotscr = pool.tile([P, D], F32)
            dotraw = pool.tile([P, 1], F32)
            for h in range(2):
                cs = slice(h * 512, (h + 1) * 512)
                nc.vector.tensor_tensor(out=dotscr[:, cs], in0=G[:, cs],
                                        in1=SpB[:, cs], op=Op.mult)
            m2 = pool.tile([P, 1], F32)
            nc.vector.tensor_tensor(out=m2[:], in0=n2[:], in1=n2spB[:],
                                    op=Op.mult)
            dotcp = pool.tile([P, D], F32)
            nc.scalar.activation(out=dotcp[:], in_=dotscr[:], func=Act.Copy,
                                 accum_out=dotraw[:])

            nrm12 = pool.tile([P, 1], F32)
            nc.scalar.activation(out=nrm12[:], in_=m2[:], func=Act.Sqrt)
            # warm Exp while the DVE norm tail runs (dep on m2 pins it here)
            nc.scalar.activation(out=wo[:, 1:2], in_=m2[0:1, 0:1],
                                 func=Act.Exp, scale=0.0)

            nmax = pool.tile([P, 1], F32)
            nc.vector.tensor_scalar_max(out=nmax[:], in0=nrm12[:],
                                        scalar1=N12MIN)
            rs2 = pool.tile([P, 1], F32)
            nc.vector.reciprocal(out=rs2[:], in_=nmax[:])
            simv = pool.tile([P, 1], F32)
            nc.vector.tensor_scalar_mul(out=simv[:], in0=dotraw[:],
                                        scalar1=rs2[:])

            # -------- stage D: per-row softmax stats + argmax -------------
            simSpread = pool.tile([P, C], F32)
            nc.vector.tensor_scalar_mul(out=simSpread[:], in0=onehotPT,
                                        scalar1=simv[:])
            simGrid = psum.tile([BL, C], F32)
            nc.tensor.matmul(out=simGrid[:], lhsT=ET, rhs=simSpread[:],
                             start=True, stop=True)
            sg = pool.tile([BL, C], F32)
            nc.vector.tensor_copy(out=sg[:], in_=simGrid[:])
            nc.vector.memset(sg[:, 0:1], NEG)

            pack = pool.tile([BL, 3], F32)
            expv = pool.tile([BL, C], F32)
            nc.scalar.activation(out=expv[:], in_=sg[:], func=Act.Exp,
                                 accum_out=pack[:, 0:1])

            selscr = pool.tile([BL, C], F32)
            nc.vector.tensor_tensor(out=selscr[:], in0=sg[:], in1=onehotF[:],
                                    op=Op.mult)
            nc.vector.tensor_reduce(out=pack[:, 1:2], in_=selscr[:],
                                    axis=mybir.AxisListType.X, op=Op.add)

            mx = pool.tile([BL, 8], F32)
            mi = pool.tile([BL, 8], U32)
            nc.vector.max_with_indices(out_max=mx[:], out_indices=mi[:],
                                       in_=sg[:])
            nc.vector.tensor_scalar(out=pack[:, 2:3], in0=mi[:, 0:1],
                                    scalar1=-1.0, scalar2=None, op0=Op.add)

            nc.sync.dma_start(out=out_d, in_=pack[:])

    nc.compile()
    return nc


_NC = None
_CONSTS = None


def _get_nc():
    global _NC
    if _NC is None:
        _NC = _build()
    return _NC


def _get_consts():
    global _CONSTS
    if _CONSTS is None:
        _CONSTS = _make_consts()
    return _CONSTS


def _make_mbig(mask_rows, lab_rows, ebf):
    """[BL, 641] bf16 = maskf | E | labels+1 for one core."""
    mb = np.empty((BL, MB_W), dtype=ml_dtypes.bfloat16)
    mb[:, 0:S] = mask_rows.astype(ml_dtypes.bfloat16)
    mb[:, S:S + P] = ebf
    mb[:, S + P] = (lab_rows.astype(np.float32) + 1.0).astype(
        ml_dtypes.bfloat16)
    return mb


def kernel(sequence_output, sot_positions, labels):
    seq = np.ascontiguousarray(np.asarray(sequence_output, dtype=np.float32))
    mask = np.asarray(sot_positions)
    lab = np.ascontiguousarray(np.asarray(labels, dtype=np.int32))
    fbig, ebf = _get_consts()

    in_maps = []
    for i in range(NCORES):
        r = slice(i * BL, (i + 1) * BL)
        in_maps.append({
            "seq": seq[r].reshape(BL * S, D),
            "mbig": _make_mbig(mask[r], lab[r], ebf),
            "fbig": fbig,
        })

    res = run_bass_kernel_spmd(_get_nc(), in_maps, core_ids=list(range(NCORES)))
    packs = np.stack([np.asarray(r["out"]) for r in res.results])  # [8, BL, 3]
    sumexp = packs[:, :, 0].reshape(-1)
    selv = packs[:, :, 1].reshape(-1)
    # per-row cross-entropy epilogue (fp32, matches log-sum-exp numerics)
    ce = np.log(sumexp, dtype=np.float32) - selv
    pred = np.rint(packs[:, :, 2]).astype(np.int32)

    valid = lab >= 0
    n_valid = np.float32(valid.sum())
    loss = np.float32(
        np.sum(np.where(valid, ce, np.float32(0.0)), dtype=np.float32)
        / max(n_valid, np.float32(1.0)))
    return loss, pred.reshape(-1), lab


# revision 20
# speedup vs baseline: 1.0618x; 1.0618x over previous
"""BertInsertion loss kernel for 8 Trainium2 NeuronCores.

Strategy: pure data parallelism over the batch (64 rows -> 8 rows/core).
Each core, on device:
  1. Finds the C=16 marked positions per row from the 0/1 sot mask
     (bf16 cumsum scan + fused count-compare, no sort).
  2. Ragged-gathers the 16 D=1024 vectors per row from HBM with an
     indirect DMA (~0.5 MB instead of the 16 MB shard); the 8 speaker
     rows are a second tiny gather issued first, then broadcast to
     their 16 partitions on the idle tensor engine.
  3. Cosine sims via multiply + row-reduce; 1/(|a||sp|) from the scalar
     engine's Sqrt (pre-warmed, stays resident: Square is table-free)
     and the exact vector-engine reciprocal.
  4. Per-row softmax statistics (sum of exps, label-selected sim) and
     the argmax prediction, packed into one [8,3] output.
Host reshards inputs, supplies input-independent constants, and runs the
loss epilogue: ce = ln(sumexp) - sim_label, then the masked mean.
"""

import numpy as np
import ml_dtypes

import concourse.bass as bass
import concourse.bacc as bacc
import concourse.tile as tile
from concourse import mybir
from concourse.bass_utils import run_bass_kernel_spmd

B, S, D, C = 64, 512, 1024, 16
NCORES = 8
BL = B // NCORES  # batch rows per core
P = BL * C        # 128 gathered vectors per core = full partition dim
N12MIN = 1e-12    # clamp for |a|*|sp|, matches eps=1e-6 per norm
NEG = -40.0       # below any cosine sim; exp(-40) ~ 4e-18, invisible in f32

F32 = mybir.dt.float32
BF16 = mybir.dt.bfloat16
I32 = mybir.dt.int32
U32 = mybir.dt.uint32
Op = mybir.AluOpType
Act = mybir.ActivationFunctionType

# fbig [128, 171] f32:
#   onehotPT(16) | ET(8) | rvals(1) | offv(1) | it16(16, rows 0:8) |
#   E(128, rows 0:8) | spioff(1, rows 0:8)
FB_W = C + BL + 1 + 1 + C + P + 1
# mbig [8, 641] bf16: maskf(512) | E(128) | labels+1(1)
MB_W = S + P + 1


def _make_consts():
    p = np.arange(P)
    b = np.arange(BL)
    onehotPT = (p[:, None] % C == np.arange(C)[None, :]).astype(np.float32)
    ET = (p[:, None] // C == b[None, :]).astype(np.float32)
    rvals = (p % C + 1).astype(np.float32)[:, None]
    offv = ((p // C) * S).astype(np.float32)[:, None]
    it16 = np.zeros((P, C), np.float32)
    it16[:BL] = np.arange(C, dtype=np.float32)[None, :]
    E = (np.arange(P)[None, :] // C == b[:, None]).astype(np.float32)
    E_pad = np.zeros((P, P), np.float32)
    E_pad[:BL] = E
    spioff = np.zeros((P, 1), np.float32)
    spioff[:BL, 0] = b * S
    fbig = np.ascontiguousarray(
        np.concatenate([onehotPT, ET, rvals, offv, it16, E_pad, spioff],
                       axis=1))
    ebf = E.astype(ml_dtypes.bfloat16)
    return fbig, ebf


def _build():
    nc = bacc.Bacc("TRN2", target_bir_lowering=False, debug=False)

    seq = nc.dram_tensor("seq", [BL * S, D], F32, kind="ExternalInput").ap()
    mbig_d = nc.dram_tensor("mbig", [BL, MB_W], BF16, kind="ExternalInput").ap()
    fbig_d = nc.dram_tensor("fbig", [P, FB_W], F32, kind="ExternalInput").ap()
    out_d = nc.dram_tensor("out", [BL, 3], F32, kind="ExternalOutput").ap()

    with tile.TileContext(nc) as tc:
        import contextlib

        with contextlib.ExitStack() as ctx:
            pool = ctx.enter_context(tc.tile_pool(name="sb", bufs=1))
            psum = ctx.enter_context(tc.tile_pool(name="ps", bufs=1, space="PSUM"))

            # -------- input DMAs: one per HWDGE queue ---------------------
            mbig = pool.tile([BL, MB_W], BF16)
            nc.sync.dma_start(out=mbig[:], in_=mbig_d)
            fbig = pool.tile([P, FB_W], F32)
            nc.scalar.dma_start(out=fbig[:], in_=fbig_d)

            m_bf = mbig[:, 0:S]
            ebf = mbig[:, S:S + P]
            lp1 = mbig[:, S + P:S + P + 1]
            c0 = 0
            onehotPT = fbig[:, c0:c0 + C]
            c0 += C
            ET = fbig[:, c0:c0 + BL]
            c0 += BL
            rvals = fbig[:, c0:c0 + 1]
            c0 += 1
            offv = fbig[:, c0:c0 + 1]
            c0 += 1
            it16 = fbig[0:BL, c0:c0 + C]
            c0 += C
            E = fbig[0:BL, c0:c0 + P]
            c0 += P
            spioff = fbig[0:BL, c0:c0 + 1]

            # -------- pre-warm Sqrt (stays resident: Square is tableless) -
            w = pool.tile([1, 1], F32)
            nc.vector.memset(w[:], 1.0)
            wo = pool.tile([1, 3], F32)
            nc.scalar.activation(out=wo[:, 0:1], in_=w[:], func=Act.Sqrt)

            # -------- stage A: mask -> gather indices ---------------------
            zeros = pool.tile([BL, S], BF16)
            nc.vector.memset(zeros[:], 0.0)
            cum = pool.tile([BL, S], BF16)  # inclusive cumsum (values <= 16)
            nc.vector.tensor_tensor_scan(out=cum[:], data0=m_bf,
                                         data1=zeros[:], initial=0.0,
                                         op0=Op.add, op1=Op.add)
            # speaker (first marked position): tiny gather issued first
            spcnt = pool.tile([BL, S], F32)
            spidx_f = pool.tile([BL, 1], F32)
            nc.vector.tensor_scalar(out=spcnt[:], in0=cum[:], scalar1=1.0,
                                    scalar2=None, op0=Op.is_lt, op1=Op.add,
                                    accum_out=spidx_f[:])
            spidx_i = pool.tile([BL, 1], I32)
            nc.vector.tensor_scalar(out=spidx_i[:], in0=spidx_f[:],
                                    scalar1=spioff, scalar2=None, op0=Op.add)
            Spk = pool.tile([BL, D], F32)
            nc.gpsimd.indirect_dma_start(
                out=Spk[:], out_offset=None, in_=seq,
                in_offset=bass.IndirectOffsetOnAxis(ap=spidx_i[:, :1], axis=0))

            cumB = psum.tile([P, S], F32)
            nc.tensor.matmul(out=cumB[:], lhsT=ebf, rhs=cum[:],
                             start=True, stop=True)
            ind = pool.tile([P, S], F32)
            idx_f = pool.tile([P, 1], F32)
            nc.vector.tensor_scalar(out=ind[:], in0=cumB[:], scalar1=rvals,
                                    scalar2=None, op0=Op.is_lt, op1=Op.add,
                                    accum_out=idx_f[:])
            idx_i = pool.tile([P, 1], I32)
            nc.vector.tensor_scalar(out=idx_i[:], in0=idx_f[:], scalar1=offv,
                                    scalar2=None, op0=Op.add)
            G = pool.tile([P, D], F32)
            nc.gpsimd.indirect_dma_start(
                out=G[:], out_offset=None, in_=seq,
                in_offset=bass.IndirectOffsetOnAxis(ap=idx_i[:, :1], axis=0))

            # label one-hot, ready long before the tail
            lp1f = pool.tile([BL, 1], F32)
            nc.vector.tensor_copy(out=lp1f[:], in_=lp1)
            onehotF = pool.tile([BL, C], F32)
            nc.vector.tensor_scalar(out=onehotF[:], in0=it16, scalar1=lp1f[:],
                                    scalar2=None, op0=Op.is_equal)

            # -------- stage C: speaker norms + broadcast (all pre-G) ------
            sqsp = pool.tile([BL, D], F32)
            n2sp = pool.tile([BL, 1], F32)
            nc.scalar.activation(out=sqsp[:], in_=Spk[:], func=Act.Square,
                                 accum_out=n2sp[:])
            n2spB = psum.tile([P, 1], F32)
            nc.tensor.matmul(out=n2spB[:], lhsT=E, rhs=n2sp[:],
                             start=True, stop=True)
            SpB = psum.tile([P, D], F32)
            for h in range(2):
                cs = slice(h * 512, (h + 1) * 512)
                nc.tensor.matmul(out=SpB[:, cs], lhsT=E, rhs=Spk[:, cs],
                                 start=True, stop=True)

            # -------- main norms + sims (post-G) --------------------------
            sq = pool.tile([P, D], F32)
            n2 = pool.tile([P, 1], F32)
            nc.scalar.activation(out=sq[:], in_=G[:], func=Act.Square,
                                 accum_out=n2[:])
            dotscr = pool.tile([P, D], F32)
            dotraw = pool.tile([P, 1], F32)
            with tc.high_priority(offset=6):
                for h in range(2):
                    cs = slice(h * 512, (h + 1) * 512)
                    nc.vector.tensor_tensor(out=dotscr[:, cs], in0=G[:, cs],
                                            in1=SpB[:, cs], op=Op.mult)
            m2 = pool.tile([P, 1], F32)
            nc.vector.tensor_tensor(out=m2[:], in0=n2[:], in1=n2spB[:],
                                    op=Op.mult)
            dotcp = pool.tile([P, D], F32)
            nc.scalar.activation(out=dotcp[:], in_=dotscr[:], func=Act.Copy,
                                 accum_out=dotraw[:])

            nrm12 = pool.tile([P, 1], F32)
            nc.scalar.activation(out=nrm12[:], in_=m2[:], func=Act.Sqrt)
            # warm Exp while the DVE norm tail runs (dep on m2 pins it here)
            nc.scalar.activation(out=wo[:, 1:2], in_=m2[0:1, 0:1],
                                 func=Act.Exp, scale=0.0)

            nmax = pool.tile([P, 1], F32)
            nc.vector.tensor_scalar_max(out=nmax[:], in0=nrm12[:],
                                        scalar1=N12MIN)
            rs2 = pool.tile([P, 1], F32)
            nc.vector.reciprocal(out=rs2[:], in_=nmax[:])
            simv = pool.tile([P, 1], F32)
            nc.vector.tensor_scalar_mul(out=simv[:], in0=dotraw[:],
                                        scalar1=rs2[:])

            # -------- stage D: per-row softmax stats + argmax -------------
            simSpread = pool.tile([P, C], F32)
            nc.vector.tensor_scalar_mul(out=simSpread[:], in0=onehotPT,
                                        scalar1=simv[:])
            simGrid = psum.tile([BL, C], F32)
            nc.tensor.matmul(out=simGrid[:], lhsT=ET, rhs=simSpread[:],
                             start=True, stop=True)
            sg = pool.tile([BL, C], F32)
            nc.vector.tensor_copy(out=sg[:], in_=simGrid[:])
            nc.vector.memset(sg[:, 0:1], NEG)

            pack = pool.tile([BL, 3], F32)
            expv = pool.tile([BL, C], F32)
            nc.scalar.activation(out=expv[:], in_=sg[:], func=Act.Exp,
                                 accum_out=pack[:, 0:1])

            selscr = pool.tile([BL, C], F32)
            nc.vector.tensor_tensor(out=selscr[:], in0=sg[:], in1=onehotF[:],
                                    op=Op.mult)
            nc.vector.tensor_reduce(out=pack[:, 1:2], in_=selscr[:],
                                    axis=mybir.AxisListType.X, op=Op.add)

            mx = pool.tile([BL, 8], F32)
            mi = pool.tile([BL, 8], U32)
            nc.vector.max_with_indices(out_max=mx[:], out_indices=mi[:],
                                       in_=sg[:])
            nc.vector.tensor_scalar(out=pack[:, 2:3], in0=mi[:, 0:1],
                                    scalar1=-1.0, scalar2=None, op0=Op.add)

            nc.sync.dma_start(out=out_d, in_=pack[:])

    nc.compile()
    return nc


_NC = None
_CONSTS = None


def _get_nc():
    global _NC
    if _NC is None:
        _NC = _build()
    return _NC


def _get_consts():
    global _CONSTS
    if _CONSTS is None:
        _CONSTS = _make_consts()
    return _CONSTS


def _make_mbig(mask_rows, lab_rows, ebf):
    """[BL, 641] bf16 = maskf | E | labels+1 for one core."""
    mb = np.empty((BL, MB_W), dtype=ml_dtypes.bfloat16)
    mb[:, 0:S] = mask_rows.astype(ml_dtypes.bfloat16)
    mb[:, S:S + P] = ebf
    mb[:, S + P] = (lab_rows.astype(np.float32) + 1.0).astype(
        ml_dtypes.bfloat16)
    return mb


def kernel(sequence_output, sot_positions, labels):
    seq = np.ascontiguousarray(np.asarray(sequence_output, dtype=np.float32))
    mask = np.asarray(sot_positions)
    lab = np.ascontiguousarray(np.asarray(labels, dtype=np.int32))
    fbig, ebf = _get_consts()

    in_maps = []
    for i in range(NCORES):
        r = slice(i * BL, (i + 1) * BL)
        in_maps.append({
            "seq": seq[r].reshape(BL * S, D),
            "mbig": _make_mbig(mask[r], lab[r], ebf),
            "fbig": fbig,
        })

    res = run_bass_kernel_spmd(_get_nc(), in_maps, core_ids=list(range(NCORES)))
    packs = np.stack([np.asarray(r["out"]) for r in res.results])  # [8, BL, 3]
    sumexp = packs[:, :, 0].reshape(-1)
    selv = packs[:, :, 1].reshape(-1)
    # per-row cross-entropy epilogue (fp32, matches log-sum-exp numerics)
    ce = np.log(sumexp, dtype=np.float32) - selv
    pred = np.rint(packs[:, :, 2]).astype(np.int32)

    valid = lab >= 0
    n_valid = np.float32(valid.sum())
    loss = np.float32(
        np.sum(np.where(valid, ce, np.float32(0.0)), dtype=np.float32)
        / max(n_valid, np.float32(1.0)))
    return loss, pred.reshape(-1), lab


# revision 21
# speedup vs baseline: 1.0931x; 1.0295x over previous
"""BertInsertion loss kernel for 8 Trainium2 NeuronCores.

Strategy: pure data parallelism over the batch (64 rows -> 8 rows/core).
Each core, on device:
  1. Finds the C=16 marked positions per row from the 0/1 sot mask
     (bf16 cumsum scan + fused count-compare, no sort).
  2. Ragged-gathers the 16 D=1024 vectors per row from HBM with an
     indirect DMA (~0.5 MB instead of the 16 MB shard); the 8 speaker
     rows are a second tiny gather issued first, then broadcast to
     their 16 partitions on the idle tensor engine.
  3. Cosine sims via multiply + row-reduce; 1/(|a||sp|) from the scalar
     engine's Sqrt (pre-warmed, stays resident: Square is table-free)
     and the exact vector-engine reciprocal.
  4. Per-row softmax statistics (sum of exps, label-selected sim) and
     the argmax prediction, packed into one [8,3] output.
Host reshards inputs, supplies input-independent constants, and runs the
loss epilogue: ce = ln(sumexp) - sim_label, then the masked mean.
"""

import numpy as np
import ml_dtypes

import concourse.bass as bass
import concourse.bacc as bacc
import concourse.tile as tile
from concourse import mybir
from concourse.bass_utils import run_bass_kernel_spmd

B, S, D, C = 64, 512, 1024, 16
NCORES = 8
BL = B // NCORES  # batch rows per core
P = BL * C        # 128 gathered vectors per core = full partition dim
N12MIN = 1e-12    # clamp for |a|*|sp|, matches eps=1e-6 per norm
NEG = -40.0       # below any cosine sim; exp(-40) ~ 4e-18, invisible in f32

F32 = mybir.dt.float32
BF16 = mybir.dt.bfloat16
I32 = mybir.dt.int32
U32 = mybir.dt.uint32
Op = mybir.AluOpType
Act = mybir.ActivationFunctionType

# fbig [128, 171] f32:
#   onehotPT(16) | ET(8) | rvals(1) | offv(1) | it16(16, rows 0:8) |
#   E(128, rows 0:8) | spioff(1, rows 0:8)
FB_W = C + BL + 1 + 1 + C + P + 1
# mbig [8, 641] bf16: maskf(512) | E(128) | labels+1(1)
MB_W = S + P + 1


def _make_consts():
    p = np.arange(P)
    b = np.arange(BL)
    onehotPT = (p[:, None] % C == np.arange(C)[None, :]).astype(np.float32)
    ET = (p[:, None] // C == b[None, :]).astype(np.float32)
    rvals = (p % C + 1).astype(np.float32)[:, None]
    offv = ((p // C) * S).astype(np.float32)[:, None]
    it16 = np.zeros((P, C), np.float32)
    it16[:BL] = np.arange(C, dtype=np.float32)[None, :]
    E = (np.arange(P)[None, :] // C == b[:, None]).astype(np.float32)
    E_pad = np.zeros((P, P), np.float32)
    E_pad[:BL] = E
    spioff = np.zeros((P, 1), np.float32)
    spioff[:BL, 0] = b * S
    fbig = np.ascontiguousarray(
        np.concatenate([onehotPT, ET, rvals, offv, it16, E_pad, spioff],
                       axis=1))
    ebf = E.astype(ml_dtypes.bfloat16)
    return fbig, ebf


def _build():
    nc = bacc.Bacc("TRN2", target_bir_lowering=False, debug=False)

    seq = nc.dram_tensor("seq", [BL * S, D], F32, kind="ExternalInput").ap()
    mbig_d = nc.dram_tensor("mbig", [BL, MB_W], BF16, kind="ExternalInput").ap()
    fbig_d = nc.dram_tensor("fbig", [P, FB_W], F32, kind="ExternalInput").ap()
    out_d = nc.dram_tensor("out", [BL, 3], F32, kind="ExternalOutput").ap()

    with tile.TileContext(nc) as tc:
        import contextlib

        with contextlib.ExitStack() as ctx:
            pool = ctx.enter_context(tc.tile_pool(name="sb", bufs=1))
            psum = ctx.enter_context(tc.tile_pool(name="ps", bufs=1, space="PSUM"))

            # -------- input DMAs: one per HWDGE queue ---------------------
            mbig = pool.tile([BL, MB_W], BF16)
            nc.sync.dma_start(out=mbig[:], in_=mbig_d)
            fbig = pool.tile([P, FB_W], F32)
            nc.scalar.dma_start(out=fbig[:], in_=fbig_d)

            m_bf = mbig[:, 0:S]
            ebf = mbig[:, S:S + P]
            lp1 = mbig[:, S + P:S + P + 1]
            c0 = 0
            onehotPT = fbig[:, c0:c0 + C]
            c0 += C
            ET = fbig[:, c0:c0 + BL]
            c0 += BL
            rvals = fbig[:, c0:c0 + 1]
            c0 += 1
            offv = fbig[:, c0:c0 + 1]
            c0 += 1
            it16 = fbig[0:BL, c0:c0 + C]
            c0 += C
            E = fbig[0:BL, c0:c0 + P]
            c0 += P
            spioff = fbig[0:BL, c0:c0 + 1]

            # -------- pre-warm Sqrt (stays resident: Square is tableless) -
            w = pool.tile([1, 1], F32)
            nc.vector.memset(w[:], 1.0)
            wo = pool.tile([1, 3], F32)
            nc.scalar.activation(out=wo[:, 0:1], in_=w[:], func=Act.Sqrt)

            # -------- stage A: mask -> gather indices ---------------------
            zeros = pool.tile([BL, S], BF16)
            nc.vector.memset(zeros[:], 0.0)
            cum = pool.tile([BL, S], BF16)  # inclusive cumsum (values <= 16)
            nc.vector.tensor_tensor_scan(out=cum[:], data0=m_bf,
                                         data1=zeros[:], initial=0.0,
                                         op0=Op.add, op1=Op.add)
            # speaker (first marked position): tiny gather issued first
            spcnt = pool.tile([BL, S], F32)
            spidx_f = pool.tile([BL, 1], F32)
            nc.vector.tensor_scalar(out=spcnt[:], in0=cum[:], scalar1=1.0,
                                    scalar2=None, op0=Op.is_lt, op1=Op.add,
                                    accum_out=spidx_f[:])
            spidx_i = pool.tile([BL, 1], I32)
            nc.vector.tensor_scalar(out=spidx_i[:], in0=spidx_f[:],
                                    scalar1=spioff, scalar2=None, op0=Op.add)
            Spk = pool.tile([BL, D], F32)
            nc.gpsimd.indirect_dma_start(
                out=Spk[:], out_offset=None, in_=seq,
                in_offset=bass.IndirectOffsetOnAxis(ap=spidx_i[:, :1], axis=0))

            cumB = psum.tile([P, S], F32)
            nc.tensor.matmul(out=cumB[:], lhsT=ebf, rhs=cum[:],
                             start=True, stop=True)
            # keep the PE busy so it ramps to 2.4 GHz before the fp32
            # speaker-broadcast matmuls (HAM warmup); results unused
            pewarm = psum.tile([P, S], F32)
            for _ in range(8):
                nc.tensor.matmul(out=pewarm[:], lhsT=ebf, rhs=cum[:],
                                 start=True, stop=True)
            ind = pool.tile([P, S], F32)
            idx_f = pool.tile([P, 1], F32)
            nc.vector.tensor_scalar(out=ind[:], in0=cumB[:], scalar1=rvals,
                                    scalar2=None, op0=Op.is_lt, op1=Op.add,
                                    accum_out=idx_f[:])
            idx_i = pool.tile([P, 1], I32)
            nc.vector.tensor_scalar(out=idx_i[:], in0=idx_f[:], scalar1=offv,
                                    scalar2=None, op0=Op.add)
            G = pool.tile([P, D], F32)
            nc.gpsimd.indirect_dma_start(
                out=G[:], out_offset=None, in_=seq,
                in_offset=bass.IndirectOffsetOnAxis(ap=idx_i[:, :1], axis=0))

            # label one-hot, ready long before the tail
            lp1f = pool.tile([BL, 1], F32)
            nc.vector.tensor_copy(out=lp1f[:], in_=lp1)
            onehotF = pool.tile([BL, C], F32)
            nc.vector.tensor_scalar(out=onehotF[:], in0=it16, scalar1=lp1f[:],
                                    scalar2=None, op0=Op.is_equal)

            # -------- stage C: speaker norms + broadcast (all pre-G) ------
            sqsp = pool.tile([BL, D], F32)
            n2sp = pool.tile([BL, 1], F32)
            nc.scalar.activation(out=sqsp[:], in_=Spk[:], func=Act.Square,
                                 accum_out=n2sp[:])
            n2spB = psum.tile([P, 1], F32)
            nc.tensor.matmul(out=n2spB[:], lhsT=E, rhs=n2sp[:],
                             start=True, stop=True)
            SpB = psum.tile([P, D], F32)
            for h in range(2):
                cs = slice(h * 512, (h + 1) * 512)
                nc.tensor.matmul(out=SpB[:, cs], lhsT=E, rhs=Spk[:, cs],
                                 start=True, stop=True)

            # -------- main norms + sims (post-G) --------------------------
            sq = pool.tile([P, D], F32)
            n2 = pool.tile([P, 1], F32)
            nc.scalar.activation(out=sq[:], in_=G[:], func=Act.Square,
                                 accum_out=n2[:])
            dotscr = pool.tile([P, D], F32)
            dotraw = pool.tile([P, 1], F32)
            with tc.high_priority(offset=6):
                for h in range(2):
                    cs = slice(h * 512, (h + 1) * 512)
                    nc.vector.tensor_tensor(out=dotscr[:, cs], in0=G[:, cs],
                                            in1=SpB[:, cs], op=Op.mult)
            m2 = pool.tile([P, 1], F32)
            nc.vector.tensor_tensor(out=m2[:], in0=n2[:], in1=n2spB[:],
                                    op=Op.mult)
            dotcp = pool.tile([P, D], F32)
            nc.scalar.activation(out=dotcp[:], in_=dotscr[:], func=Act.Copy,
                                 accum_out=dotraw[:])

            nrm12 = pool.tile([P, 1], F32)
            nc.scalar.activation(out=nrm12[:], in_=m2[:], func=Act.Sqrt)
            # warm Exp while the DVE norm tail runs (dep on m2 pins it here)
            nc.scalar.activation(out=wo[:, 1:2], in_=m2[0:1, 0:1],
                                 func=Act.Exp, scale=0.0)

            nmax = pool.tile([P, 1], F32)
            nc.vector.tensor_scalar_max(out=nmax[:], in0=nrm12[:],
                                        scalar1=N12MIN)
            rs2 = pool.tile([P, 1], F32)
            nc.vector.reciprocal(out=rs2[:], in_=nmax[:])
            simv = pool.tile([P, 1], F32)
            nc.vector.tensor_scalar_mul(out=simv[:], in0=dotraw[:],
                                        scalar1=rs2[:])

            # -------- stage D: per-row softmax stats + argmax -------------
            simSpread = pool.tile([P, C], F32)
            nc.vector.tensor_scalar_mul(out=simSpread[:], in0=onehotPT,
                                        scalar1=simv[:])
            simGrid = psum.tile([BL, C], F32)
            nc.tensor.matmul(out=simGrid[:], lhsT=ET, rhs=simSpread[:],
                             start=True, stop=True)
            sg = pool.tile([BL, C], F32)
            nc.vector.tensor_copy(out=sg[:], in_=simGrid[:])
            nc.vector.memset(sg[:, 0:1], NEG)

            pack = pool.tile([BL, 3], F32)
            expv = pool.tile([BL, C], F32)
            nc.scalar.activation(out=expv[:], in_=sg[:], func=Act.Exp,
                                 accum_out=pack[:, 0:1])

            selscr = pool.tile([BL, C], F32)
            nc.vector.tensor_tensor(out=selscr[:], in0=sg[:], in1=onehotF[:],
                                    op=Op.mult)
            nc.vector.tensor_reduce(out=pack[:, 1:2], in_=selscr[:],
                                    axis=mybir.AxisListType.X, op=Op.add)

            mx = pool.tile([BL, 8], F32)
            mi = pool.tile([BL, 8], U32)
            nc.vector.max_with_indices(out_max=mx[:], out_indices=mi[:],
                                       in_=sg[:])
            nc.vector.tensor_scalar(out=pack[:, 2:3], in0=mi[:, 0:1],
                                    scalar1=-1.0, scalar2=None, op0=Op.add)

            nc.sync.dma_start(out=out_d, in_=pack[:])

    nc.compile()
    return nc


_NC = None
_CONSTS = None


def _get_nc():
    global _NC
    if _NC is None:
        _NC = _build()
    return _NC


def _get_consts():
    global _CONSTS
    if _CONSTS is None:
        _CONSTS = _make_consts()
    return _CONSTS


def _make_mbig(mask_rows, lab_rows, ebf):
    """[BL, 641] bf16 = maskf | E | labels+1 for one core."""
    mb = np.empty((BL, MB_W), dtype=ml_dtypes.bfloat16)
    mb[:, 0:S] = mask_rows.astype(ml_dtypes.bfloat16)
    mb[:, S:S + P] = ebf
    mb[:, S + P] = (lab_rows.astype(np.float32) + 1.0).astype(
        ml_dtypes.bfloat16)
    return mb


def kernel(sequence_output, sot_positions, labels):
    seq = np.ascontiguousarray(np.asarray(sequence_output, dtype=np.float32))
    mask = np.asarray(sot_positions)
    lab = np.ascontiguousarray(np.asarray(labels, dtype=np.int32))
    fbig, ebf = _get_consts()

    in_maps = []
    for i in range(NCORES):
        r = slice(i * BL, (i + 1) * BL)
        in_maps.append({
            "seq": seq[r].reshape(BL * S, D),
            "mbig": _make_mbig(mask[r], lab[r], ebf),
            "fbig": fbig,
        })

    res = run_bass_kernel_spmd(_get_nc(), in_maps, core_ids=list(range(NCORES)))
    packs = np.stack([np.asarray(r["out"]) for r in res.results])  # [8, BL, 3]
    sumexp = packs[:, :, 0].reshape(-1)
    selv = packs[:, :, 1].reshape(-1)
    # per-row cross-entropy epilogue (fp32, matches log-sum-exp numerics)
    ce = np.log(sumexp, dtype=np.float32) - selv
    pred = np.rint(packs[:, :, 2]).astype(np.int32)

    valid = lab >= 0
    n_valid = np.float32(valid.sum())
    loss = np.float32(
        np.sum(np.where(valid, ce, np.float32(0.0)), dtype=np.float32)
        / max(n_valid, np.float32(1.0)))
    return loss, pred.reshape(-1), lab


# revision 22
# speedup vs baseline: 1.0958x; 1.0025x over previous
"""BertInsertion loss kernel for 8 Trainium2 NeuronCores.

Strategy: pure data parallelism over the batch (64 rows -> 8 rows/core).
Each core, on device:
  1. Finds the C=16 marked positions per row from the 0/1 sot mask
     (bf16 cumsum scan + fused count-compare, no sort).
  2. Ragged-gathers the 16 D=1024 vectors per row from HBM with an
     indirect DMA (~0.5 MB instead of the 16 MB shard); the 8 speaker
     rows are a second tiny gather issued first, then broadcast to
     their 16 partitions on the idle tensor engine.
  3. Cosine sims via multiply + row-reduce; 1/(|a||sp|) from the scalar
     engine's Sqrt (pre-warmed, stays resident: Square is table-free)
     and the exact vector-engine reciprocal.
  4. Per-row softmax statistics (sum of exps, label-selected sim) and
     the argmax prediction, packed into one [8,3] output.
Host reshards inputs, supplies input-independent constants, and runs the
loss epilogue: ce = ln(sumexp) - sim_label, then the masked mean.
"""

import numpy as np
import ml_dtypes

import concourse.bass as bass
import concourse.bacc as bacc
import concourse.tile as tile
from concourse import mybir
from concourse.bass_utils import run_bass_kernel_spmd

B, S, D, C = 64, 512, 1024, 16
NCORES = 8
BL = B // NCORES  # batch rows per core
P = BL * C        # 128 gathered vectors per core = full partition dim
N12MIN = 1e-12    # clamp for |a|*|sp|, matches eps=1e-6 per norm
NEG = -40.0       # below any cosine sim; exp(-40) ~ 4e-18, invisible in f32

F32 = mybir.dt.float32
BF16 = mybir.dt.bfloat16
I32 = mybir.dt.int32
U32 = mybir.dt.uint32
Op = mybir.AluOpType
Act = mybir.ActivationFunctionType

# fbig [128, 171] f32:
#   onehotPT(16) | ET(8) | rvals(1) | offv(1) | it16(16, rows 0:8) |
#   E(128, rows 0:8) | spioff(1, rows 0:8)
FB_W = C + BL + 1 + 1 + C + P + 1
# mbig [8, 641] bf16: maskf(512) | E(128) | labels+1(1)
MB_W = S + P + 1


def _make_consts():
    p = np.arange(P)
    b = np.arange(BL)
    onehotPT = (p[:, None] % C == np.arange(C)[None, :]).astype(np.float32)
    ET = (p[:, None] // C == b[None, :]).astype(np.float32)
    rvals = (p % C + 1).astype(np.float32)[:, None]
    offv = ((p // C) * S).astype(np.float32)[:, None]
    it16 = np.zeros((P, C), np.float32)
    it16[:BL] = np.arange(C, dtype=np.float32)[None, :]
    E = (np.arange(P)[None, :] // C == b[:, None]).astype(np.float32)
    E_pad = np.zeros((P, P), np.float32)
    E_pad[:BL] = E
    spioff = np.zeros((P, 1), np.float32)
    spioff[:BL, 0] = b * S
    fbig = np.ascontiguousarray(
        np.concatenate([onehotPT, ET, rvals, offv, it16, E_pad, spioff],
                       axis=1))
    ebf = E.astype(ml_dtypes.bfloat16)
    return fbig, ebf


def _build():
    nc = bacc.Bacc("TRN2", target_bir_lowering=False, debug=False)

    seq = nc.dram_tensor("seq", [BL * S, D], F32, kind="ExternalInput").ap()
    mbig_d = nc.dram_tensor("mbig", [BL, MB_W], BF16, kind="ExternalInput").ap()
    fbig_d = nc.dram_tensor("fbig", [P, FB_W], F32, kind="ExternalInput").ap()
    out_d = nc.dram_tensor("out", [BL, 3], F32, kind="ExternalOutput").ap()

    with tile.TileContext(nc) as tc:
        import contextlib

        with contextlib.ExitStack() as ctx:
            pool = ctx.enter_context(tc.tile_pool(name="sb", bufs=1))
            psum = ctx.enter_context(tc.tile_pool(name="ps", bufs=1, space="PSUM"))

            # -------- input DMAs: one per HWDGE queue ---------------------
            mbig = pool.tile([BL, MB_W], BF16)
            nc.sync.dma_start(out=mbig[:], in_=mbig_d)
            fbig = pool.tile([P, FB_W], F32)
            nc.scalar.dma_start(out=fbig[:], in_=fbig_d)

            m_bf = mbig[:, 0:S]
            ebf = mbig[:, S:S + P]
            lp1 = mbig[:, S + P:S + P + 1]
            c0 = 0
            onehotPT = fbig[:, c0:c0 + C]
            c0 += C
            ET = fbig[:, c0:c0 + BL]
            c0 += BL
            rvals = fbig[:, c0:c0 + 1]
            c0 += 1
            offv = fbig[:, c0:c0 + 1]
            c0 += 1
            it16 = fbig[0:BL, c0:c0 + C]
            c0 += C
            E = fbig[0:BL, c0:c0 + P]
            c0 += P
            spioff = fbig[0:BL, c0:c0 + 1]

            # -------- pre-warm Sqrt (stays resident: Square is tableless) -
            w = pool.tile([1, 1], F32)
            nc.vector.memset(w[:], 1.0)
            wo = pool.tile([1, 3], F32)
            nc.scalar.activation(out=wo[:, 0:1], in_=w[:], func=Act.Sqrt)

            # -------- stage A: mask -> gather indices ---------------------
            zeros = pool.tile([BL, S], BF16)
            nc.vector.memset(zeros[:], 0.0)
            cum = pool.tile([BL, S], BF16)  # inclusive cumsum (values <= 16)
            nc.vector.tensor_tensor_scan(out=cum[:], data0=m_bf,
                                         data1=zeros[:], initial=0.0,
                                         op0=Op.add, op1=Op.add)
            cumB = psum.tile([P, S], F32)
            nc.tensor.matmul(out=cumB[:], lhsT=ebf, rhs=cum[:],
                             start=True, stop=True)
            # keep the PE busy so it ramps to 2.4 GHz before the fp32
            # speaker-broadcast matmuls (HAM warmup); results unused
            pewarm = psum.tile([P, S], F32)
            for _ in range(8):
                nc.tensor.matmul(out=pewarm[:], lhsT=ebf, rhs=cum[:],
                                 start=True, stop=True)
            ind = pool.tile([P, S], F32)
            idx_f = pool.tile([P, 1], F32)
            nc.vector.tensor_scalar(out=ind[:], in0=cumB[:], scalar1=rvals,
                                    scalar2=None, op0=Op.is_lt, op1=Op.add,
                                    accum_out=idx_f[:])
            idx_i = pool.tile([P, 1], I32)
            nc.vector.tensor_scalar(out=idx_i[:], in0=idx_f[:], scalar1=offv,
                                    scalar2=None, op0=Op.add)
            G = pool.tile([P, D], F32)
            nc.gpsimd.indirect_dma_start(
                out=G[:], out_offset=None, in_=seq,
                in_offset=bass.IndirectOffsetOnAxis(ap=idx_i[:, :1], axis=0))

            # speaker (first marked position): tiny gather issued after G
            spcnt = pool.tile([BL, S], F32)
            spidx_f = pool.tile([BL, 1], F32)
            nc.vector.tensor_scalar(out=spcnt[:], in0=cum[:], scalar1=1.0,
                                    scalar2=None, op0=Op.is_lt, op1=Op.add,
                                    accum_out=spidx_f[:])
            spidx_i = pool.tile([BL, 1], I32)
            nc.vector.tensor_scalar(out=spidx_i[:], in0=spidx_f[:],
                                    scalar1=spioff, scalar2=None, op0=Op.add)
            Spk = pool.tile([BL, D], F32)
            nc.gpsimd.indirect_dma_start(
                out=Spk[:], out_offset=None, in_=seq,
                in_offset=bass.IndirectOffsetOnAxis(ap=spidx_i[:, :1], axis=0))

            # label one-hot, ready long before the tail
            lp1f = pool.tile([BL, 1], F32)
            nc.vector.tensor_copy(out=lp1f[:], in_=lp1)
            onehotF = pool.tile([BL, C], F32)
            nc.vector.tensor_scalar(out=onehotF[:], in0=it16, scalar1=lp1f[:],
                                    scalar2=None, op0=Op.is_equal)

            # -------- stage C: speaker norms + broadcast (all pre-G) ------
            sqsp = pool.tile([BL, D], F32)
            n2sp = pool.tile([BL, 1], F32)
            nc.scalar.activation(out=sqsp[:], in_=Spk[:], func=Act.Square,
                                 accum_out=n2sp[:])
            n2spB = psum.tile([P, 1], F32)
            nc.tensor.matmul(out=n2spB[:], lhsT=E, rhs=n2sp[:],
                             start=True, stop=True)
            SpB = psum.tile([P, D], F32)
            for h in range(2):
                cs = slice(h * 512, (h + 1) * 512)
                nc.tensor.matmul(out=SpB[:, cs], lhsT=E, rhs=Spk[:, cs],
                                 start=True, stop=True)

            # -------- main norms + sims (post-G) --------------------------
            sq = pool.tile([P, D], F32)
            n2 = pool.tile([P, 1], F32)
            nc.scalar.activation(out=sq[:], in_=G[:], func=Act.Square,
                                 accum_out=n2[:])
            dotscr = pool.tile([P, D], F32)
            dotraw = pool.tile([P, 1], F32)
            with tc.high_priority(offset=6):
                for h in range(2):
                    cs = slice(h * 512, (h + 1) * 512)
                    nc.vector.tensor_tensor(out=dotscr[:, cs], in0=G[:, cs],
                                            in1=SpB[:, cs], op=Op.mult)
            m2 = pool.tile([P, 1], F32)
            nc.vector.tensor_tensor(out=m2[:], in0=n2[:], in1=n2spB[:],
                                    op=Op.mult)
            dotcp = pool.tile([P, D], F32)
            nc.scalar.activation(out=dotcp[:], in_=dotscr[:], func=Act.Copy,
                                 accum_out=dotraw[:])

            nrm12 = pool.tile([P, 1], F32)
            nc.scalar.activation(out=nrm12[:], in_=m2[:], func=Act.Sqrt)
            # warm Exp while the DVE norm tail runs (dep on m2 pins it here)
            nc.scalar.activation(out=wo[:, 1:2], in_=m2[0:1, 0:1],
                                 func=Act.Exp, scale=0.0)

            nmax = pool.tile([P, 1], F32)
            nc.vector.tensor_scalar_max(out=nmax[:], in0=nrm12[:],
                                        scalar1=N12MIN)
            rs2 = pool.tile([P, 1], F32)
            nc.vector.reciprocal(out=rs2[:], in_=nmax[:])
            simv = pool.tile([P, 1], F32)
            nc.vector.tensor_scalar_mul(out=simv[:], in0=dotraw[:],
                                        scalar1=rs2[:])

            # -------- stage D: per-row softmax stats + argmax -------------
            simSpread = pool.tile([P, C], F32)
            nc.vector.tensor_scalar_mul(out=simSpread[:], in0=onehotPT,
                                        scalar1=simv[:])
            simGrid = psum.tile([BL, C], F32)
            nc.tensor.matmul(out=simGrid[:], lhsT=ET, rhs=simSpread[:],
                             start=True, stop=True)
            sg = pool.tile([BL, C], F32)
            nc.vector.tensor_copy(out=sg[:], in_=simGrid[:])
            nc.vector.memset(sg[:, 0:1], NEG)

            pack = pool.tile([BL, 3], F32)
            expv = pool.tile([BL, C], F32)
            nc.scalar.activation(out=expv[:], in_=sg[:], func=Act.Exp,
                                 accum_out=pack[:, 0:1])

            selscr = pool.tile([BL, C], F32)
            nc.vector.tensor_tensor(out=selscr[:], in0=sg[:], in1=onehotF[:],
                                    op=Op.mult)
            nc.vector.tensor_reduce(out=pack[:, 1:2], in_=selscr[:],
                                    axis=mybir.AxisListType.X, op=Op.add)

            mx = pool.tile([BL, 8], F32)
            mi = pool.tile([BL, 8], U32)
            nc.vector.max_with_indices(out_max=mx[:], out_indices=mi[:],
                                       in_=sg[:])
            nc.vector.tensor_scalar(out=pack[:, 2:3], in0=mi[:, 0:1],
                                    scalar1=-1.0, scalar2=None, op0=Op.add)

            nc.sync.dma_start(out=out_d, in_=pack[:])

    nc.compile()
    return nc


_NC = None
_CONSTS = None


def _get_nc():
    global _NC
    if _NC is None:
        _NC = _build()
    return _NC


def _get_consts():
    global _CONSTS
    if _CONSTS is None:
        _CONSTS = _make_consts()
    return _CONSTS


def _make_mbig(mask_rows, lab_rows, ebf):
    """[BL, 641] bf16 = maskf | E | labels+1 for one core."""
    mb = np.empty((BL, MB_W), dtype=ml_dtypes.bfloat16)
    mb[:, 0:S] = mask_rows.astype(ml_dtypes.bfloat16)
    mb[:, S:S + P] = ebf
    mb[:, S + P] = (lab_rows.astype(np.float32) + 1.0).astype(
        ml_dtypes.bfloat16)
    return mb


def kernel(sequence_output, sot_positions, labels):
    seq = np.ascontiguousarray(np.asarray(sequence_output, dtype=np.float32))
    mask = np.asarray(sot_positions)
    lab = np.ascontiguousarray(np.asarray(labels, dtype=np.int32))
    fbig, ebf = _get_consts()

    in_maps = []
    for i in range(NCORES):
        r = slice(i * BL, (i + 1) * BL)
        in_maps.append({
            "seq": seq[r].reshape(BL * S, D),
            "mbig": _make_mbig(mask[r], lab[r], ebf),
            "fbig": fbig,
        })

    res = run_bass_kernel_spmd(_get_nc(), in_maps, core_ids=list(range(NCORES)))
    packs = np.stack([np.asarray(r["out"]) for r in res.results])  # [8, BL, 3]
    sumexp = packs[:, :, 0].reshape(-1)
    selv = packs[:, :, 1].reshape(-1)
    # per-row cross-entropy epilogue (fp32, matches log-sum-exp numerics)
    ce = np.log(sumexp, dtype=np.float32) - selv
    pred = np.rint(packs[:, :, 2]).astype(np.int32)

    valid = lab >= 0
    n_valid = np.float32(valid.sum())
    loss = np.float32(
        np.sum(np.where(valid, ce, np.float32(0.0)), dtype=np.float32)
        / max(n_valid, np.float32(1.0)))
    return loss, pred.reshape(-1), lab
